# revision 1
# baseline (speedup 1.0000x reference)
"""Deformable conv (nn_DeformConv) Trainium2 Bass kernel.

Sharding: 8 cores = 4 batches x 2 H-halves (spatial). Each core computes
out[b, :, h0:h0+32, :] for its half.

Per-core pipeline:
  1. offset conv (fp32r GEMM over shifted slices of the pad-1 image)
  2. PE-transpose offsets to [pixel-part, (blk, ch)] layout
  3. DVE index/alpha math (floor via int-cast, bilinear weights)
  4. PE "fold" of fp32 indices into the SWDGE idx wrap layout, cast int16
  5. dma_gather (fp16, overlapping x-pair elems) from the padded transposed
     image xT [9216, 256]
  6. pass-1: bilinear combine + transpose via diagonal matmuls into PSUM
  7. pass-2: grouped GEMM (fp32r) accumulating 9 taps into PSUM
"""

import dataclasses
import numpy as np
from contextlib import ExitStack

import concourse.bacc as bacc
import concourse.mybir as mybir
from concourse.tile import TileContext
from concourse.library_config import mlp
from concourse.bass_utils import run_bass_kernel_spmd

F16 = mybir.dt.float16
F32 = mybir.dt.float32
F32R = mybir.dt.float32r
I16 = mybir.dt.int16
I32 = mybir.dt.int32
OP = mybir.AluOpType

B, C, H, W = 4, 192, 64, 64
K, KK, G = 3, 9, 4
HP = 32                # output rows per core
PIX = HP * W           # 2048 pixels per core
NBLK = PIX // 128      # 16 pixel blocks of 128
NCH = 4                # 512-pixel chunks
PADG = 16              # gather-image pad on each side
GDIM = H + 2 * PADG    # 96
GCH = 256              # padded channel count in xT rows
NROWS = GDIM * GDIM    # 9216
WRAPM = (2 * KK * PIX) // 16  # 2304 wrap columns

_CACHE = {}


def _replace_ap(ap, new_free_dims, extra_offset=0):
    return dataclasses.replace(
        ap, ap=[ap.ap[0]] + [list(d) for d in new_free_dims],
        offset=ap.offset + extra_offset,
    )


def _build_nc():
    nc = bacc.Bacc("TRN2", target_bir_lowering=False)

    xT_h = nc.dram_tensor("xT", [NROWS, GCH], F16, kind="ExternalInput")
    xc_h = nc.dram_tensor("xc", [2, 96, 34, 66], F32, kind="ExternalInput")
    wofft_h = nc.dram_tensor("wofft", [96, 324], F32, kind="ExternalInput")
    boff_h = nc.dram_tensor("boff", [18, 1], F32, kind="ExternalInput")
    wdeft_h = nc.dram_tensor("wdeft", [128, 1728], F32, kind="ExternalInput")
    ident_h = nc.dram_tensor("ident", [128, 128], F32, kind="ExternalInput")
    maskh_h = nc.dram_tensor("maskh", [128, 128], F16, kind="ExternalInput")
    selfold_h = nc.dram_tensor("selfold", [128, 256], F32, kind="ExternalInput")
    baseY_h = nc.dram_tensor("baseY", [128, 16, 9], F32, kind="ExternalInput")
    baseX_h = nc.dram_tensor("baseX", [128, 16, 9], F32, kind="ExternalInput")
    out_h = nc.dram_tensor("out", [192, PIX], F32, kind="ExternalOutput")

    with TileContext(nc) as tc:
        nc.gpsimd.load_library(mlp)
        with ExitStack() as ctx:
            cpool = ctx.enter_context(tc.tile_pool(name="const", bufs=1))

            xc_s = cpool.tile([96, 2, 34, 66], F32)
            nc.sync.dma_start(xc_s[:, 0], xc_h[0])
            nc.sync.dma_start(xc_s[:, 1], xc_h[1])
            wofft_s = cpool.tile([96, 324], F32)
            nc.sync.dma_start(wofft_s[:], wofft_h[:])
            boff_s = cpool.tile([18, 1], F32)
            nc.sync.dma_start(boff_s[:], boff_h[:])
            wdeft_s = cpool.tile([128, 1728], F32)
            nc.sync.dma_start(wdeft_s[:], wdeft_h[:])
            ident_s = cpool.tile([128, 128], F32)
            nc.sync.dma_start(ident_s[:], ident_h[:])
            maskh_s = cpool.tile([128, 128], F16)
            nc.sync.dma_start(maskh_s[:], maskh_h[:])
            selfold_s = cpool.tile([128, 256], F32)
            nc.sync.dma_start(selfold_s[:], selfold_h[:])
            baseY_s = cpool.tile([128, 16, 9], F32)
            nc.sync.dma_start(baseY_s[:], baseY_h[:])
            baseX_s = cpool.tile([128, 16, 9], F32)
            nc.sync.dma_start(baseX_s[:], baseX_h[:])

            # fp32r-rounded copies (fp32r matmul operands must be produced
            # by a rounding instruction)
            xcr_s = cpool.tile([96, 2, 34, 66], F32R)
            nc.vector.tensor_copy(xcr_s[:], xc_s[:])
            woffr_s = cpool.tile([96, 324], F32R)
            nc.vector.tensor_copy(woffr_s[:], wofft_s[:])
            wdefr_s = cpool.tile([128, 1728], F32R)
            nc.vector.tensor_copy(wdefr_s[:], wdeft_s[:])

            # persistent across main loop
            a_s = [cpool.tile([128, 16, 9], F32, name=f"alpha{j}") for j in range(4)]
            wrap_s = cpool.tile([128, WRAPM], I16)
            nc.vector.memset(wrap_s[:], 0)

            # ---------- prologue ----------
            with ExitStack() as pctx:
                ppool = pctx.enter_context(tc.tile_pool(name="prol", bufs=1))
                ppsum_off = pctx.enter_context(
                    tc.tile_pool(name="ppso", bufs=2, space="PSUM"))
                ppsum_t = pctx.enter_context(
                    tc.tile_pool(name="ppst", bufs=2, space="PSUM"))
                ppsum_f = pctx.enter_context(
                    tc.tile_pool(name="ppsf", bufs=2, space="PSUM"))

                # Stage A: offset conv -> offs_s [18, 2048]
                offs_s = ppool.tile([18, PIX], F32)
                for nch in range(NCH):
                    pco = ppsum_off.tile([18, 512], F32, name=f"pco{nch}",
                                         tag="pco", bufs=2)
                    first = True
                    for u in range(2):
                        for k in range(KK):
                            ky, kx = k // 3, k % 3
                            rhs = xcr_s[:, u, nch * 8 + ky: nch * 8 + ky + 8,
                                        kx: kx + 64]
                            lhsT = woffr_s[:, (u * 9 + k) * 18:(u * 9 + k + 1) * 18]
                            nc.tensor.matmul(
                                pco[:], lhsT, rhs,
                                start=first, stop=(u == 1 and k == KK - 1))
                            first = False
                    nc.vector.tensor_scalar(
                        offs_s[:, nch * 512:(nch + 1) * 512], pco[:],
                        boff_s[:, 0:1], None, OP.add)

                # Stage B: transpose -> offT_s [128, (blk, 18)]
                offT_s = ppool.tile([128, 16 * 18], F32)
                for blk in range(NBLK):
                    pt = ppsum_t.tile([128, 18], F32, name=f"pt{blk}",
                                      tag="pt", bufs=2)
                    nc.tensor.transpose(
                        pt[:], offs_s[:, blk * 128:(blk + 1) * 128],
                        ident_s[0:18, 0:18])
                    nc.scalar.copy(offT_s[:, blk * 18:(blk + 1) * 18], pt[:])

                # Stage C: index/alpha math on [128, (16, 9)]
                base = offT_s[:, :]
                dy = _replace_ap(base, [[18, 16], [2, 9]])
                dx = _replace_ap(base, [[18, 16], [2, 9]], extra_offset=1)

                def floorfrac(src_off, base_s, clamp_hi, nm):
                    v = ppool.tile([128, 16, 9], F32, name=f"v{nm}")
                    nc.vector.tensor_tensor(v[:], src_off, base_s[:], OP.add)
                    nc.vector.tensor_scalar(v[:], v[:], 1.5, clamp_hi,
                                            OP.max, OP.min)
                    vi = ppool.tile([128, 16, 9], I32, name=f"vi{nm}")
                    nc.vector.tensor_copy(vi[:], v[:])
                    vr = ppool.tile([128, 16, 9], F32, name=f"vr{nm}")
                    nc.vector.tensor_copy(vr[:], vi[:])
                    corr = ppool.tile([128, 16, 9], F32, name=f"corr{nm}")
                    nc.vector.tensor_tensor(corr[:], vr[:], v[:], OP.is_gt)
                    v0 = ppool.tile([128, 16, 9], F32, name=f"v0{nm}")
                    nc.vector.tensor_tensor(v0[:], vr[:], corr[:], OP.subtract)
                    fr = ppool.tile([128, 16, 9], F32, name=f"fr{nm}")
                    nc.vector.tensor_tensor(fr[:], v[:], v0[:], OP.subtract)
                    return v0, fr

                y0p, fy = floorfrac(dy, baseY_s, 94.5, "y")
                x0p, fx = floorfrac(dx, baseX_s, 93.5, "x")
                gy = ppool.tile([128, 16, 9], F32)
                nc.vector.tensor_scalar(gy[:], fy[:], -1.0, 1.0, OP.mult, OP.add)
                gx = ppool.tile([128, 16, 9], F32)
                nc.vector.tensor_scalar(gx[:], fx[:], -1.0, 1.0, OP.mult, OP.add)
                for j, (ta, tb) in enumerate([(gy, gx), (gy, fx), (fy, gx), (fy, fx)]):
                    nc.vector.tensor_tensor(a_s[j][:], ta[:], tb[:], OP.mult)

                # idxf [128, (c4, k9, t2, s4)] fp32; STT/TS are limited to
                # 2 free dims, so emit per 512-chunk.
                idxf = ppool.tile([128, 288], F32)
                for c4 in range(4):
                    srcd = [[1, 9], [9, 4]]
                    y0_ap = _replace_ap(y0p[:, :], srcd, extra_offset=c4 * 36)
                    x0_ap = _replace_ap(x0p[:, :], srcd, extra_offset=c4 * 36)
                    dstd = [[8, 9], [1, 4]]
                    idxA = _replace_ap(idxf[:, :], dstd, extra_offset=c4 * 72)
                    idxB = _replace_ap(idxf[:, :], dstd, extra_offset=c4 * 72 + 4)
                    nc.vector.scalar_tensor_tensor(
                        idxA, y0_ap, float(GDIM), x0_ap, OP.mult, OP.add)
                    nc.vector.tensor_scalar(idxB, idxA, float(GDIM), None, OP.add)

                # Stage D: fold into wrap layout, cast int16
                for piece in range(6):
                    pf = ppsum_f.tile([32, 384], F32, name=f"pf{piece}",
                                      tag="pf", bufs=2)
                    for phi in range(8):
                        nc.tensor.matmul(
                            pf[:, phi * 48:(phi + 1) * 48],
                            selfold_s[:, phi * 32: phi * 32 + 32],
                            idxf[:, piece * 48:(piece + 1) * 48],
                            start=True, stop=True)
                    dst = _replace_ap(wrap_s[0:32, :], [[8, 48], [1, 8]],
                                      extra_offset=piece * 384)
                    src = _replace_ap(pf[:, :], [[1, 48], [48, 8]])
                    nc.scalar.copy(dst, src)

            # ---------- main loop ----------
            with ExitStack() as mctx:
                gpool = mctx.enter_context(tc.tile_pool(name="gat", bufs=2))
                dpool = mctx.enter_context(tc.tile_pool(name="diag", bufs=2))
                stpool = mctx.enter_context(tc.tile_pool(name="stg", bufs=2))
                obpool = mctx.enter_context(tc.tile_pool(name="ob", bufs=2))
                ps1pool = mctx.enter_context(
                    tc.tile_pool(name="ps1", bufs=2, space="PSUM"))
                popool = mctx.enter_context(
                    tc.tile_pool(name="po", bufs=2, space="PSUM"))

                pair_ap = dataclasses.replace(
                    xT_h[:, :], ap=[[GCH, NROWS - 1], [1, 2 * GCH]])

                for cch in range(NCH):
                    pos = [popool.tile([64, 512], F32, name=f"po{cch}_{g}",
                                       tag=f"po{g}", bufs=1) for g in range(4)]
                    for k in range(KK):
                        gt = gpool.tile([128, 8, 2 * GCH], F16,
                                        name=f"gt{cch}_{k}", tag="gt", bufs=2)
                        callc = (cch * 9 + k) * 64
                        nc.gpsimd.dma_gather(
                            gt[:], pair_ap, wrap_s[:, callc: callc + 64],
                            1024, 1024, 2 * GCH, elem_step=GCH)
                        stg = stpool.tile([128, 1024], F32R, name=f"stg{cch}_{k}",
                                          tag="stg", bufs=2)
                        for s in range(4):
                            blk = cch * 4 + s
                            ps1 = ps1pool.tile([128, 256], F32,
                                               name=f"ps1_{cch}_{k}_{s}",
                                               tag="ps1", bufs=2)
                            dgs = []
                            for j in range(4):
                                dg = dpool.tile([128, 128], F16,
                                                name=f"dg{cch}_{k}_{s}_{j}",
                                                tag=f"dg{j}", bufs=2)
                                nc.vector.tensor_scalar(
                                    dg[:], maskh_s[:],
                                    a_s[j][:, blk, k: k + 1], None, OP.mult)
                                dgs.append(dg)
                            cA = [gt[:, s, 0:128], gt[:, s, GCH: GCH + 128],
                                  gt[:, 4 + s, 0:128], gt[:, 4 + s, GCH: GCH + 128]]
                            cB = [gt[:, s, 128:256], gt[:, s, GCH + 128: GCH + 256],
                                  gt[:, 4 + s, 128:256],
                                  gt[:, 4 + s, GCH + 128: GCH + 256]]
                            for j in range(4):
                                nc.tensor.matmul(ps1[:, 0:128], cA[j], dgs[j][:],
                                                 start=(j == 0), stop=(j == 3))
                            for j in range(4):
                                nc.tensor.matmul(ps1[:, 128:256], cB[j], dgs[j][:],
                                                 start=(j == 0), stop=(j == 3))
                            nc.scalar.copy(stg[:, s * 128:(s + 1) * 128],
                                           ps1[:, 0:128])
                            nc.scalar.copy(stg[:, 512 + s * 128: 512 + (s + 1) * 128],
                                           ps1[:, 128:256])  # B-half
                        # pass-2: 4 groups, fp32r, N=512
                        for g in range(4):
                            half = g // 2
                            rb = 64 * (g % 2)
                            lhsT = wdefr_s[rb: rb + 64,
                                           (g * 9 + k) * 48:(g * 9 + k + 1) * 48]
                            nc.tensor.matmul(
                                pos[g][0:48, :],
                                lhsT,
                                stg[rb: rb + 64, half * 512:(half + 1) * 512],
                                start=(k == 0), stop=(k == KK - 1))
                    ob = obpool.tile([48, 4, 512], F32, name=f"ob{cch}",
                                     tag="ob", bufs=2)
                    cs = slice(cch * 512, (cch + 1) * 512)
                    for g in range(4):
                        cp = (nc.scalar.copy if g % 2 == 0
                              else nc.vector.tensor_copy)
                        cp(ob[:, g, :], pos[g][0:48, :])
                        nc.sync.dma_start(out_h[g * 48:(g + 1) * 48, cs],
                                          ob[:, g, :])

    nc.compile()
    return nc


def _host_inputs(x, w_off, b_off, w_def):
    """Per-core input dicts (8 cores = 4 batches x 2 halves)."""
    x = np.ascontiguousarray(x, np.float32)
    ky = np.repeat(np.arange(K), K).astype(np.float32)
    kx = np.tile(np.arange(K), K).astype(np.float32)

    # gather image: [b, 9216, 256] fp16
    xT = np.zeros((B, GDIM, GDIM, GCH), np.float16)
    xv = x.transpose(0, 2, 3, 1)
    for g in range(G):
        xT[:, PADG:PADG + H, PADG:PADG + W, 64 * g:64 * g + 48] = \
            xv[:, :, :, 48 * g:48 * (g + 1)]
    xT = xT.reshape(B, NROWS, GCH)

    # conv image: pad-1, [b, 192, 66, 66]
    xc_full = np.zeros((B, C, 66, 66), np.float32)
    xc_full[:, :, 1:65, 1:65] = x

    wofft = np.zeros((96, 324), np.float32)
    for u in range(2):
        for k in range(KK):
            wofft[:, (u * 9 + k) * 18:(u * 9 + k + 1) * 18] = \
                w_off[:, u * 96:(u + 1) * 96, k // 3, k % 3].T
    wdeft = np.zeros((128, 1728), np.float32)
    for g in range(G):
        rb = 64 * (g % 2)
        for k in range(KK):
            wdeft[rb: rb + 48, (g * 9 + k) * 48:(g * 9 + k + 1) * 48] = \
                w_def[g * 48:(g + 1) * 48, :, k // 3, k % 3].T

    ident = np.eye(128, dtype=np.float32)
    maskh = np.eye(128, dtype=np.float16)
    selfold = np.zeros((128, 256), np.float32)
    for phi in range(8):
        for j in range(32):
            selfold[16 * phi + (j % 16), phi * 32 + j] = 1.0
    boff = np.asarray(b_off, np.float32).reshape(18, 1)

    p = np.arange(128)
    blkv = np.arange(16)
    baseX = ((p % 64)[:, None, None] - 1 + kx[None, None, :] + PADG
             + np.zeros((1, 16, 1))).astype(np.float32)

    in_maps = []
    for core in range(8):
        b, half = core // 2, core % 2
        h0 = half * HP
        hloc = h0 + blkv[None, :, None] * 2 + (p[:, None, None] // 64)
        baseY = (hloc - 1 + ky[None, None, :] + PADG).astype(np.float32)
        xc = np.zeros((2, 96, 34, 66), np.float32)
        win = xc_full[b, :, h0:h0 + 34, :]          # rows h0-1..h0+32 padded
        xc[0] = win[0:96]
        xc[1] = win[96:192]
        in_maps.append({
            "xT": xT[b], "xc": xc, "wofft": wofft, "boff": boff,
            "wdeft": wdeft, "ident": ident, "maskh": maskh,
            "selfold": selfold,
            "baseY": baseY, "baseX": np.ascontiguousarray(baseX),
        })
    return in_maps


def kernel(x, w_off, b_off, w_def):
    if "nc" not in _CACHE:
        _CACHE["nc"] = _build_nc()
    nc = _CACHE["nc"]
    in_maps = _host_inputs(np.asarray(x), np.asarray(w_off),
                           np.asarray(b_off), np.asarray(w_def))
    res = run_bass_kernel_spmd(nc, in_maps, core_ids=list(range(8)))
    out = np.zeros((B, C, H, W), np.float32)
    for core in range(8):
        b, half = core // 2, core % 2
        out[b, :, half * HP:(half + 1) * HP, :] = \
            res.results[core]["out"].reshape(C, HP, W)
    return out



# revision 7
# speedup vs baseline: 1.2993x; 1.2993x over previous
"""Deformable conv (nn_DeformConv) Trainium2 Bass kernel.

Sharding: 8 cores = 4 batches x 2 H-halves (spatial). Each core computes
out[b, :, h0:h0+32, :] for its half.

Per-core pipeline:
  1. offset conv (fp32r GEMM over shifted slices of the pad-1 image)
  2. PE-transpose offsets to [pixel-part, (blk, ch)] layout
  3. DVE index/alpha math (floor via int-cast, bilinear weights)
  4. PE "fold" of fp32 indices into the SWDGE idx wrap layout, cast int16
  5. dma_gather (fp16, overlapping x-pair elems) from the padded transposed
     image xT [9216, 256]
  6. pass-1: bilinear combine + transpose via diagonal matmuls into PSUM
  7. pass-2: grouped GEMM (fp32r) accumulating 9 taps into PSUM
"""

import dataclasses
import numpy as np
from contextlib import ExitStack

import concourse.bacc as bacc
import concourse.mybir as mybir
from concourse.tile import TileContext
from concourse.library_config import mlp
from concourse.bass_utils import run_bass_kernel_spmd

F16 = mybir.dt.float16
F32 = mybir.dt.float32
F32R = mybir.dt.float32r
I16 = mybir.dt.int16
I32 = mybir.dt.int32
OP = mybir.AluOpType

B, C, H, W = 4, 192, 64, 64
K, KK, G = 3, 9, 4
HP = 32                # output rows per core
PIX = HP * W           # 2048 pixels per core
NBLK = PIX // 128      # 16 pixel blocks of 128
NCH = 4                # 512-pixel chunks
PADG = 16              # gather-image pad on each side
GDIM = H + 2 * PADG    # 96
GCH = 256              # padded channel count in xT rows
NROWS = GDIM * GDIM    # 9216
WRAPM = (KK * PIX) // 16  # 1152 wrap columns (one idx per pixel*tap)

_CACHE = {}


def _replace_ap(ap, new_free_dims, extra_offset=0):
    return dataclasses.replace(
        ap, ap=[ap.ap[0]] + [list(d) for d in new_free_dims],
        offset=ap.offset + extra_offset,
    )


def _build_nc():
    nc = bacc.Bacc("TRN2", target_bir_lowering=False)

    # xT rows are y-pair concatenated: row r = [ch(y,x), ch(y+1,x)] so one
    # gather descriptor (2 consecutive rows) covers all 4 bilinear corners.
    xT_h = nc.dram_tensor("xT", [NROWS, 2 * GCH], F16, kind="ExternalInput")
    xc_h = nc.dram_tensor("xc", [2, 96, 34, 66], F32, kind="ExternalInput")
    wofft_h = nc.dram_tensor("wofft", [96, 324], F32, kind="ExternalInput")
    boff_h = nc.dram_tensor("boff", [18, 1], F32, kind="ExternalInput")
    wdeft_h = nc.dram_tensor("wdeft", [128, 1728], F32, kind="ExternalInput")
    ident_h = nc.dram_tensor("ident", [128, 128], F32, kind="ExternalInput")
    maskh_h = nc.dram_tensor("maskh", [128, 128], F16, kind="ExternalInput")
    selfold_h = nc.dram_tensor("selfold", [128, 256], F32, kind="ExternalInput")
    baseY_h = nc.dram_tensor("baseY", [128, 16, 9], F32, kind="ExternalInput")
    baseX_h = nc.dram_tensor("baseX", [128, 16, 9], F32, kind="ExternalInput")
    out_h = nc.dram_tensor("out", [192, PIX], F32, kind="ExternalOutput")

    with TileContext(nc) as tc:
        nc.gpsimd.load_library(mlp)
        with ExitStack() as ctx:
            cpool = ctx.enter_context(tc.tile_pool(name="const", bufs=1))

            maskh_s = cpool.tile([128, 128], F16)
            nc.sync.dma_start(maskh_s[:], maskh_h[:])
            wdefr_s = cpool.tile([128, 1728], F32R)

            # persistent across main loop
            a_s = [cpool.tile([128, 16, 9], F32, name=f"alpha{j}") for j in range(4)]
            wrap_s = cpool.tile([128, WRAPM], I16)
            nc.vector.memset(wrap_s[:], 0)

            # ---------- prologue ----------
            with ExitStack() as pctx:
                ppool = pctx.enter_context(tc.tile_pool(name="prol", bufs=1))
                ppsum_off = pctx.enter_context(
                    tc.tile_pool(name="ppso", bufs=2, space="PSUM"))
                ppsum_t = pctx.enter_context(
                    tc.tile_pool(name="ppst", bufs=2, space="PSUM"))
                ppsum_f = pctx.enter_context(
                    tc.tile_pool(name="ppsf", bufs=2, space="PSUM"))

                xc_s = ppool.tile([96, 2, 34, 66], F32)
                nc.sync.dma_start(xc_s[:, 0], xc_h[0])
                nc.sync.dma_start(xc_s[:, 1], xc_h[1])
                wofft_s = ppool.tile([96, 324], F32)
                nc.sync.dma_start(wofft_s[:], wofft_h[:])
                boff_s = ppool.tile([18, 1], F32)
                nc.sync.dma_start(boff_s[:], boff_h[:])
                wdeft_s = ppool.tile([128, 1728], F32)
                nc.sync.dma_start(wdeft_s[:], wdeft_h[:])
                ident_s = ppool.tile([128, 128], F32)
                nc.sync.dma_start(ident_s[:], ident_h[:])
                selfold_s = ppool.tile([128, 256], F32)
                nc.sync.dma_start(selfold_s[:], selfold_h[:])
                baseY_s = ppool.tile([128, 16, 9], F32)
                nc.sync.dma_start(baseY_s[:], baseY_h[:])
                baseX_s = ppool.tile([128, 16, 9], F32)
                nc.sync.dma_start(baseX_s[:], baseX_h[:])

                # fp32r-rounded copies (fp32r matmul operands must be
                # produced by a rounding instruction)
                xcr_s = ppool.tile([96, 2, 34, 66], F32R)
                nc.vector.tensor_copy(xcr_s[:], xc_s[:])
                woffr_s = ppool.tile([96, 324], F32R)
                nc.vector.tensor_copy(woffr_s[:], wofft_s[:])
                nc.vector.tensor_copy(wdefr_s[:], wdeft_s[:])

                # Stage A: offset conv -> offs_s [18, 2048]
                offs_s = ppool.tile([18, PIX], F32)
                for nch in range(NCH):
                    pco = ppsum_off.tile([18, 512], F32, name=f"pco{nch}",
                                         tag="pco", bufs=2)
                    first = True
                    for u in range(2):
                        for k in range(KK):
                            ky, kx = k // 3, k % 3
                            rhs = xcr_s[:, u, nch * 8 + ky: nch * 8 + ky + 8,
                                        kx: kx + 64]
                            lhsT = woffr_s[:, (u * 9 + k) * 18:(u * 9 + k + 1) * 18]
                            nc.tensor.matmul(
                                pco[:], lhsT, rhs,
                                start=first, stop=(u == 1 and k == KK - 1))
                            first = False
                    nc.vector.tensor_scalar(
                        offs_s[:, nch * 512:(nch + 1) * 512], pco[:],
                        boff_s[:, 0:1], None, OP.add)

                # Stage B: transpose -> offT_s [128, (blk, 18)]
                offT_s = ppool.tile([128, 16 * 18], F32)
                for blk in range(NBLK):
                    pt = ppsum_t.tile([128, 18], F32, name=f"pt{blk}",
                                      tag="pt", bufs=2)
                    nc.tensor.transpose(
                        pt[:], offs_s[:, blk * 128:(blk + 1) * 128],
                        ident_s[0:18, 0:18])
                    nc.scalar.copy(offT_s[:, blk * 18:(blk + 1) * 18], pt[:])

                # Stage C: index/alpha math on [128, (16, 9)]
                base = offT_s[:, :]
                dy = _replace_ap(base, [[18, 16], [2, 9]])
                dx = _replace_ap(base, [[18, 16], [2, 9]], extra_offset=1)

                def floorfrac(src_off, base_s, clamp_hi, nm):
                    v = ppool.tile([128, 16, 9], F32, name=f"v{nm}")
                    nc.vector.tensor_tensor(v[:], src_off, base_s[:], OP.add)
                    nc.vector.tensor_scalar(v[:], v[:], 1.5, clamp_hi,
                                            OP.max, OP.min)
                    vi = ppool.tile([128, 16, 9], I32, name=f"vi{nm}")
                    nc.vector.tensor_copy(vi[:], v[:])
                    vr = ppool.tile([128, 16, 9], F32, name=f"vr{nm}")
                    nc.vector.tensor_copy(vr[:], vi[:])
                    corr = ppool.tile([128, 16, 9], F32, name=f"corr{nm}")
                    nc.vector.tensor_tensor(corr[:], vr[:], v[:], OP.is_gt)
                    v0 = ppool.tile([128, 16, 9], F32, name=f"v0{nm}")
                    nc.vector.tensor_tensor(v0[:], vr[:], corr[:], OP.subtract)
                    fr = ppool.tile([128, 16, 9], F32, name=f"fr{nm}")
                    nc.vector.tensor_tensor(fr[:], v[:], v0[:], OP.subtract)
                    return v0, fr

                y0p, fy = floorfrac(dy, baseY_s, 94.5, "y")
                x0p, fx = floorfrac(dx, baseX_s, 93.5, "x")
                gy = ppool.tile([128, 16, 9], F32)
                nc.vector.tensor_scalar(gy[:], fy[:], -1.0, 1.0, OP.mult, OP.add)
                gx = ppool.tile([128, 16, 9], F32)
                nc.vector.tensor_scalar(gx[:], fx[:], -1.0, 1.0, OP.mult, OP.add)
                for j, (ta, tb) in enumerate([(gy, gx), (gy, fx), (fy, gx), (fy, fx)]):
                    nc.vector.tensor_tensor(a_s[j][:], ta[:], tb[:], OP.mult)

                # idxf [128, (c4, k9, s4)] fp32 — one idx per (pixel, tap)
                # (row of the y-pair image; the 4-corner elem covers the rest)
                idxf = ppool.tile([128, 144], F32)
                for c4 in range(4):
                    srcd = [[1, 9], [9, 4]]
                    y0_ap = _replace_ap(y0p[:, :], srcd, extra_offset=c4 * 36)
                    x0_ap = _replace_ap(x0p[:, :], srcd, extra_offset=c4 * 36)
                    dstd = [[4, 9], [1, 4]]
                    idxA = _replace_ap(idxf[:, :], dstd, extra_offset=c4 * 36)
                    nc.vector.scalar_tensor_tensor(
                        idxA, y0_ap, float(GDIM), x0_ap, OP.mult, OP.add)

                # Stage D: fold into wrap layout, cast int16.
                # wrap[w, (cch*9+k)*32 + 8*q + e] = idx(pix=cch*512+q*128
                # +e*16+w, tap k) = idxf[e*16+w, cch*36+4k+q], dup at w+16.
                for cch in range(4):
                    pf = ppsum_f.tile([32, 8, 36], F32, name=f"pf{cch}",
                                      tag="pf", bufs=2)
                    for e in range(8):
                        nc.tensor.matmul(
                            pf[:, e, :],
                            selfold_s[:, e * 32: e * 32 + 32],
                            idxf[:, cch * 36:(cch + 1) * 36],
                            start=True, stop=True)
                    for k in range(KK):
                        dst = _replace_ap(wrap_s[0:32, :], [[8, 4], [1, 8]],
                                          extra_offset=(cch * 9 + k) * 32)
                        src = _replace_ap(pf[:, :, :], [[1, 4], [36, 8]],
                                          extra_offset=4 * k)
                        nc.scalar.copy(dst, src)

            # ---------- main loop ----------
            with ExitStack() as mctx:
                gpool = mctx.enter_context(tc.tile_pool(name="gat", bufs=2))
                dpool = mctx.enter_context(tc.tile_pool(name="diag", bufs=2))
                stpool = mctx.enter_context(tc.tile_pool(name="stg", bufs=2))
                obpool = mctx.enter_context(tc.tile_pool(name="ob", bufs=2))
                ps1pool = mctx.enter_context(
                    tc.tile_pool(name="ps1", bufs=2, space="PSUM"))
                popool = mctx.enter_context(
                    tc.tile_pool(name="po", bufs=2, space="PSUM"))

                pair_ap = dataclasses.replace(
                    xT_h[:, :], ap=[[2 * GCH, NROWS - 1], [1, 4 * GCH]])
                # corner j -> offset in the 1024-elem 4-corner gather row:
                # [0:256]=(y0,x0) [256:512]=(y1,x0) [512:768]=(y0,x1)
                # [768:1024]=(y1,x1); alpha order j = 00,01,10,11
                joff = [0, 512, 256, 768]

                for cch in range(NCH):
                    pos = [popool.tile([64, 512], F32, name=f"po{cch}_{g}",
                                       tag=f"po{g}", bufs=1) for g in range(4)]
                    for k in range(KK):
                        gt = gpool.tile([128, 4, 4 * GCH], F16,
                                        name=f"gt{cch}_{k}", tag="gt", bufs=2)
                        callc = (cch * 9 + k) * 32
                        nc.gpsimd.dma_gather(
                            gt[:], pair_ap, wrap_s[:, callc: callc + 32],
                            512, 512, 4 * GCH, elem_step=2 * GCH)
                        stg = stpool.tile([128, 1024], F32R, name=f"stg{cch}_{k}",
                                          tag="stg", bufs=2)
                        for s in range(4):
                            blk = cch * 4 + s
                            ps1 = ps1pool.tile([128, 256], F32,
                                               name=f"ps1_{cch}_{k}_{s}",
                                               tag="ps1", bufs=2)
                            dgs = []
                            for j in range(4):
                                dg = dpool.tile([128, 128], F16,
                                                name=f"dg{cch}_{k}_{s}_{j}",
                                                tag=f"dg{j}", bufs=2)
                                nc.vector.tensor_scalar(
                                    dg[:], maskh_s[:],
                                    a_s[j][:, blk, k: k + 1], None, OP.mult)
                                dgs.append(dg)
                            cA = [gt[:, s, joff[j]: joff[j] + 128]
                                  for j in range(4)]
                            cB = [gt[:, s, joff[j] + 128: joff[j] + 256]
                                  for j in range(4)]
                            for j in range(4):
                                nc.tensor.matmul(ps1[:, 0:128], cA[j], dgs[j][:],
                                                 start=(j == 0), stop=(j == 3))
                            for j in range(4):
                                nc.tensor.matmul(ps1[:, 128:256], cB[j], dgs[j][:],
                                                 start=(j == 0), stop=(j == 3))
                            nc.scalar.copy(stg[:, s * 128:(s + 1) * 128],
                                           ps1[:, 0:128])
                            nc.vector.tensor_copy(
                                stg[:, 512 + s * 128: 512 + (s + 1) * 128],
                                ps1[:, 128:256])  # B-half on DVE
                        # pass-2: 4 groups, fp32r, N=512
                        for g in range(4):
                            half = g // 2
                            rb = 64 * (g % 2)
                            lhsT = wdefr_s[rb: rb + 64,
                                           (g * 9 + k) * 48:(g * 9 + k + 1) * 48]
                            nc.tensor.matmul(
                                pos[g][0:48, :],
                                lhsT,
                                stg[rb: rb + 64, half * 512:(half + 1) * 512],
                                start=(k == 0), stop=(k == KK - 1))
                    ob = obpool.tile([48, 4, 512], F32, name=f"ob{cch}",
                                     tag="ob", bufs=2)
                    cs = slice(cch * 512, (cch + 1) * 512)
                    for g in range(4):
                        cp = (nc.scalar.copy if g % 2 == 0
                              else nc.vector.tensor_copy)
                        cp(ob[:, g, :], pos[g][0:48, :])
                        nc.sync.dma_start(out_h[g * 48:(g + 1) * 48, cs],
                                          ob[:, g, :])

    nc.compile()
    return nc


def _host_inputs(x, w_off, b_off, w_def):
    """Per-core input dicts (8 cores = 4 batches x 2 halves)."""
    x = np.ascontiguousarray(x, np.float32)
    ky = np.repeat(np.arange(K), K).astype(np.float32)
    kx = np.tile(np.arange(K), K).astype(np.float32)

    # gather image: [b, 9216, 512] fp16, row r = [ch(y,x), ch(y+1,x)]
    xT = np.zeros((B, GDIM, GDIM, GCH), np.float16)
    xv = x.transpose(0, 2, 3, 1)
    for g in range(G):
        xT[:, PADG:PADG + H, PADG:PADG + W, 64 * g:64 * g + 48] = \
            xv[:, :, :, 48 * g:48 * (g + 1)]
    xT = xT.reshape(B, NROWS, GCH)
    xT2 = np.zeros((B, NROWS, 2 * GCH), np.float16)
    xT2[:, :, :GCH] = xT
    xT2[:, :NROWS - GDIM, GCH:] = xT[:, GDIM:]
    xT = xT2

    # conv image: pad-1, [b, 192, 66, 66]
    xc_full = np.zeros((B, C, 66, 66), np.float32)
    xc_full[:, :, 1:65, 1:65] = x

    wofft = np.zeros((96, 324), np.float32)
    for u in range(2):
        for k in range(KK):
            wofft[:, (u * 9 + k) * 18:(u * 9 + k + 1) * 18] = \
                w_off[:, u * 96:(u + 1) * 96, k // 3, k % 3].T
    wdeft = np.zeros((128, 1728), np.float32)
    for g in range(G):
        rb = 64 * (g % 2)
        for k in range(KK):
            wdeft[rb: rb + 48, (g * 9 + k) * 48:(g * 9 + k + 1) * 48] = \
                w_def[g * 48:(g + 1) * 48, :, k // 3, k % 3].T

    ident = np.eye(128, dtype=np.float32)
    maskh = np.eye(128, dtype=np.float16)
    selfold = np.zeros((128, 256), np.float32)
    for phi in range(8):
        for j in range(32):
            selfold[16 * phi + (j % 16), phi * 32 + j] = 1.0
    boff = np.asarray(b_off, np.float32).reshape(18, 1)

    p = np.arange(128)
    blkv = np.arange(16)
    baseX = ((p % 64)[:, None, None] - 1 + kx[None, None, :] + PADG
             + np.zeros((1, 16, 1))).astype(np.float32)

    in_maps = []
    for core in range(8):
        b, half = core // 2, core % 2
        h0 = half * HP
        hloc = h0 + blkv[None, :, None] * 2 + (p[:, None, None] // 64)
        baseY = (hloc - 1 + ky[None, None, :] + PADG).astype(np.float32)
        xc = np.zeros((2, 96, 34, 66), np.float32)
        win = xc_full[b, :, h0:h0 + 34, :]          # rows h0-1..h0+32 padded
        xc[0] = win[0:96]
        xc[1] = win[96:192]
        in_maps.append({
            "xT": xT[b], "xc": xc, "wofft": wofft, "boff": boff,
            "wdeft": wdeft, "ident": ident, "maskh": maskh,
            "selfold": selfold,
            "baseY": baseY, "baseX": np.ascontiguousarray(baseX),
        })
    return in_maps


def kernel(x, w_off, b_off, w_def):
    if "nc" not in _CACHE:
        _CACHE["nc"] = _build_nc()
    nc = _CACHE["nc"]
    in_maps = _host_inputs(np.asarray(x), np.asarray(w_off),
                           np.asarray(b_off), np.asarray(w_def))
    res = run_bass_kernel_spmd(nc, in_maps, core_ids=list(range(8)))
    out = np.zeros((B, C, H, W), np.float32)
    for core in range(8):
        b, half = core // 2, core % 2
        out[b, :, half * HP:(half + 1) * HP, :] = \
            res.results[core]["out"].reshape(C, HP, W)
    return out



# revision 21
# speedup vs baseline: 1.4261x; 1.0976x over previous
"""Deformable conv (nn_DeformConv) Trainium2 Bass kernel.

Sharding: 8 cores = 4 batches x 2 H-halves (spatial). Each core computes
out[b, :, h0:h0+32, :] for its half.

Per-core pipeline:
  1. offset conv (fp32r GEMM over shifted slices of the pad-1 image)
  2. PE-transpose offsets to [pixel-part, (blk, ch)] layout
  3. DVE index/alpha math (floor via int-cast, bilinear weights)
  4. PE "fold" of fp32 indices into the SWDGE idx wrap layout, cast int16
  5. dma_gather (fp16, overlapping x-pair elems) from the padded transposed
     image xT [9216, 256]
  6. pass-1: bilinear combine + transpose via diagonal matmuls into PSUM
  7. pass-2: grouped GEMM (fp32r) accumulating 9 taps into PSUM
"""

import dataclasses
import numpy as np
from contextlib import ExitStack

import concourse.bacc as bacc
import concourse.mybir as mybir
from concourse.tile import TileContext
from concourse.library_config import mlp
from concourse.bass_utils import run_bass_kernel_spmd

F16 = mybir.dt.float16
BF16 = mybir.dt.bfloat16
F32 = mybir.dt.float32
F32R = mybir.dt.float32r
I16 = mybir.dt.int16
I32 = mybir.dt.int32
OP = mybir.AluOpType

B, C, H, W = 4, 192, 64, 64
K, KK, G = 3, 9, 4
HP = 32                # output rows per core
PIX = HP * W           # 2048 pixels per core
NBLK = PIX // 128      # 16 pixel blocks of 128
NCH = 4                # 512-pixel chunks
PADG = 16              # gather-image pad on each side
GDIM = H + 2 * PADG    # 96
GCH = 256              # padded channel count in xT rows
NROWS = GDIM * GDIM    # 9216
WRAPM = (KK * PIX) // 16  # 1152 wrap columns (one idx per pixel*tap)

_CACHE = {}


def _to_bf16(a):
    import ml_dtypes
    return np.asarray(a, dtype=ml_dtypes.bfloat16)


def _replace_ap(ap, new_free_dims, extra_offset=0):
    return dataclasses.replace(
        ap, ap=[ap.ap[0]] + [list(d) for d in new_free_dims],
        offset=ap.offset + extra_offset,
    )


def _build_nc():
    # 2 SWDGE queues so consecutive gathers don't share a descriptor ring
    # (ring holds 64 data descs/engine -> at most 1024 idxs per gather).
    nc = bacc.Bacc("TRN2", target_bir_lowering=False, num_swdge_queues=2)

    # xT rows are y-pair concatenated: row r = [ch(y,x), ch(y+1,x)] so one
    # gather descriptor (2 consecutive rows) covers all 4 bilinear corners.
    xT_h = nc.dram_tensor("xT", [NROWS, 2 * GCH], F16, kind="ExternalInput")
    xc_h = nc.dram_tensor("xc", [2, 96, 34, 66], F32, kind="ExternalInput")
    wofft_h = nc.dram_tensor("wofft", [96, 324], F32, kind="ExternalInput")
    boff_h = nc.dram_tensor("boff", [18, 1], F32, kind="ExternalInput")
    wdeft_h = nc.dram_tensor("wdeft", [128, 1728], F32, kind="ExternalInput")
    ident_h = nc.dram_tensor("ident", [128, 128], F32, kind="ExternalInput")
    maskh_h = nc.dram_tensor("maskh", [128, 128], F16, kind="ExternalInput")
    selfold_h = nc.dram_tensor("selfold", [128, 512], F32, kind="ExternalInput")
    baseY_h = nc.dram_tensor("baseY", [128, 16, 9], F32, kind="ExternalInput")
    baseX_h = nc.dram_tensor("baseX", [128, 16, 9], F32, kind="ExternalInput")
    out_h = nc.dram_tensor("out", [192, PIX], F32, kind="ExternalOutput")

    with TileContext(nc) as tc:
        nc.gpsimd.load_library(mlp)
        with ExitStack() as ctx:
            cpool = ctx.enter_context(tc.tile_pool(name="const", bufs=1))

            maskh_s = cpool.tile([128, 128], F16)
            nc.sync.dma_start(maskh_s[:], maskh_h[:])
            wdefr_s = cpool.tile([128, 1728], F32R)

            # persistent across main loop
            a_s = [cpool.tile([128, 16, 9], F32, name=f"alpha{j}") for j in range(4)]
            wrap_s = cpool.tile([128, WRAPM], I16)
            nc.vector.memset(wrap_s[:], 0)

            # ---------- prologue ----------
            with ExitStack() as pctx:
                ppool = pctx.enter_context(tc.tile_pool(name="prol", bufs=1))
                ppsum_off = pctx.enter_context(
                    tc.tile_pool(name="ppso", bufs=2, space="PSUM"))
                ppsum_t = pctx.enter_context(
                    tc.tile_pool(name="ppst", bufs=2, space="PSUM"))
                ppsum_f = pctx.enter_context(
                    tc.tile_pool(name="ppsf", bufs=2, space="PSUM"))

                xc_s = ppool.tile([96, 2, 34, 66], F32)
                nc.sync.dma_start(xc_s[:, 0], xc_h[0])
                nc.sync.dma_start(xc_s[:, 1], xc_h[1])
                wofft_s = ppool.tile([96, 324], F32)
                nc.sync.dma_start(wofft_s[:], wofft_h[:])
                boff_s = ppool.tile([18, 1], F32)
                nc.sync.dma_start(boff_s[:], boff_h[:])
                wdeft_s = ppool.tile([128, 1728], F32)
                nc.sync.dma_start(wdeft_s[:], wdeft_h[:])
                ident_s = ppool.tile([128, 128], F32)
                nc.sync.dma_start(ident_s[:], ident_h[:])
                selfold_s = ppool.tile([128, 512], F32)
                nc.sync.dma_start(selfold_s[:], selfold_h[:])
                baseY_s = ppool.tile([128, 16, 9], F32)
                nc.sync.dma_start(baseY_s[:], baseY_h[:])
                baseX_s = ppool.tile([128, 16, 9], F32)
                nc.sync.dma_start(baseX_s[:], baseX_h[:])

                # fp32r-rounded copies (fp32r matmul operands must be
                # produced by a rounding instruction)
                xcr_s = ppool.tile([96, 2, 34, 66], F32R)
                nc.vector.tensor_copy(xcr_s[:], xc_s[:])
                woffr_s = ppool.tile([96, 324], F32R)
                nc.vector.tensor_copy(woffr_s[:], wofft_s[:])
                nc.vector.tensor_copy(wdefr_s[:], wdeft_s[:])

                # Stage A: offset conv -> offs_s [18, 2048]
                offs_s = ppool.tile([18, PIX], F32)
                for nch in range(NCH):
                    pco = ppsum_off.tile([18, 512], F32, name=f"pco{nch}",
                                         tag="pco", bufs=2)
                    first = True
                    for u in range(2):
                        for k in range(KK):
                            ky, kx = k // 3, k % 3
                            rhs = xcr_s[:, u, nch * 8 + ky: nch * 8 + ky + 8,
                                        kx: kx + 64]
                            lhsT = woffr_s[:, (u * 9 + k) * 18:(u * 9 + k + 1) * 18]
                            nc.tensor.matmul(
                                pco[:], lhsT, rhs,
                                start=first, stop=(u == 1 and k == KK - 1))
                            first = False
                    nc.vector.tensor_scalar(
                        offs_s[:, nch * 512:(nch + 1) * 512], pco[:],
                        boff_s[:, 0:1], None, OP.add)

                # Stage B: transpose -> offT_s [128, (blk, 18)]
                offT_s = ppool.tile([128, 16 * 18], F32)
                for blk in range(NBLK):
                    pt = ppsum_t.tile([128, 18], F32, name=f"pt{blk}",
                                      tag="pt", bufs=2)
                    nc.tensor.transpose(
                        pt[:], offs_s[:, blk * 128:(blk + 1) * 128],
                        ident_s[0:18, 0:18])
                    nc.scalar.copy(offT_s[:, blk * 18:(blk + 1) * 18], pt[:])

                # Stage C: index/alpha math on [128, (16, 9)]
                base = offT_s[:, :]
                dy = _replace_ap(base, [[18, 16], [2, 9]])
                dx = _replace_ap(base, [[18, 16], [2, 9]], extra_offset=1)

                def floorfrac(src_off, base_s, clamp_hi, nm):
                    v = ppool.tile([128, 16, 9], F32, name=f"v{nm}")
                    nc.vector.tensor_tensor(v[:], src_off, base_s[:], OP.add)
                    nc.vector.tensor_scalar(v[:], v[:], 1.5, clamp_hi,
                                            OP.max, OP.min)
                    vi = ppool.tile([128, 16, 9], I32, name=f"vi{nm}")
                    nc.vector.tensor_copy(vi[:], v[:])
                    vr = ppool.tile([128, 16, 9], F32, name=f"vr{nm}")
                    nc.vector.tensor_copy(vr[:], vi[:])
                    corr = ppool.tile([128, 16, 9], F32, name=f"corr{nm}")
                    nc.vector.tensor_tensor(corr[:], vr[:], v[:], OP.is_gt)
                    v0 = ppool.tile([128, 16, 9], F32, name=f"v0{nm}")
                    nc.vector.tensor_tensor(v0[:], vr[:], corr[:], OP.subtract)
                    fr = ppool.tile([128, 16, 9], F32, name=f"fr{nm}")
                    nc.vector.tensor_tensor(fr[:], v[:], v0[:], OP.subtract)
                    return v0, fr

                y0p, fy = floorfrac(dy, baseY_s, 94.5, "y")
                x0p, fx = floorfrac(dx, baseX_s, 93.5, "x")
                gy = ppool.tile([128, 16, 9], F32)
                nc.vector.tensor_scalar(gy[:], fy[:], -1.0, 1.0, OP.mult, OP.add)
                gx = ppool.tile([128, 16, 9], F32)
                nc.vector.tensor_scalar(gx[:], fx[:], -1.0, 1.0, OP.mult, OP.add)
                for j, (ta, tb) in enumerate([(gy, gx), (gy, fx), (fy, gx), (fy, fx)]):
                    nc.vector.tensor_tensor(a_s[j][:], ta[:], tb[:], OP.mult)

                # idxf [128, (c4, k9, s4)] fp32 — one idx per (pixel, tap)
                # (row of the y-pair image; the 4-corner elem covers the rest)
                idxf = ppool.tile([128, 144], F32)
                for c4 in range(4):
                    srcd = [[1, 9], [9, 4]]
                    y0_ap = _replace_ap(y0p[:, :], srcd, extra_offset=c4 * 36)
                    x0_ap = _replace_ap(x0p[:, :], srcd, extra_offset=c4 * 36)
                    dstd = [[4, 9], [1, 4]]
                    idxA = _replace_ap(idxf[:, :], dstd, extra_offset=c4 * 36)
                    nc.vector.scalar_tensor_tensor(
                        idxA, y0_ap, float(GDIM), x0_ap, OP.mult, OP.add)

                # Stage D: fold into wrap layout, cast int16.
                # wrap[w, (cch*9+k)*32 + 8*q + e] = idx(pix=cch*512+q*128
                # +e*16+w, tap k) = idxf[e*16+w, cch*36+4k+q], dup at w+16.
                for cch in range(4):
                    pf = ppsum_f.tile([64, 8, 36], F32, name=f"pf{cch}",
                                      tag="pf", bufs=2)
                    for e in range(8):
                        nc.tensor.matmul(
                            pf[:, e, :],
                            selfold_s[:, e * 64: e * 64 + 64],
                            idxf[:, cch * 36:(cch + 1) * 36],
                            start=True, stop=True)
                    for k in range(KK):
                        dst = _replace_ap(wrap_s[0:64, :], [[8, 4], [1, 8]],
                                          extra_offset=(cch * 9 + k) * 32)
                        src = _replace_ap(pf[:, :, :], [[1, 4], [36, 8]],
                                          extra_offset=4 * k)
                        nc.scalar.copy(dst, src)

            # ---------- main loop ----------
            with ExitStack() as mctx:
                gpool = mctx.enter_context(tc.tile_pool(name="gat", bufs=2))
                dpool = mctx.enter_context(tc.tile_pool(name="diag", bufs=2))
                stpool = mctx.enter_context(tc.tile_pool(name="stg", bufs=2))
                obpool = mctx.enter_context(tc.tile_pool(name="ob", bufs=2))
                ps1pool = mctx.enter_context(
                    tc.tile_pool(name="ps1", bufs=2, space="PSUM"))
                popool = mctx.enter_context(
                    tc.tile_pool(name="po", bufs=2, space="PSUM"))

                pair_ap = dataclasses.replace(
                    xT_h[:, :], ap=[[2 * GCH, NROWS - 1], [1, 4 * GCH]])
                # corner j -> offset in the 1024-elem 4-corner gather row:
                # [0:256]=(y0,x0) [256:512]=(y1,x0) [512:768]=(y0,x1)
                # [768:1024]=(y1,x1); alpha order j = 00,01,10,11
                joff = [0, 512, 256, 768]

                for cch in range(NCH):
                    pos = [popool.tile([64, 512], F32, name=f"po{cch}_{g}",
                                       tag=f"po{g}", bufs=1) for g in range(4)]
                    for kb, taps in enumerate([2, 2, 2, 2, 1]):
                        k0 = 2 * kb
                        gt = gpool.tile([128, 4 * taps, 4 * GCH], F16,
                                        name=f"gt{cch}_{kb}",
                                        tag=f"gt{taps}", bufs=2)
                        callc = (cch * 9 + k0) * 32
                        nc.gpsimd.dma_gather(
                            gt[:], pair_ap,
                            wrap_s[:, callc: callc + 32 * taps],
                            512 * taps, 512 * taps, 4 * GCH,
                            elem_step=2 * GCH,
                            queue_num=(cch * 5 + kb) % 2)
                        for dk in range(taps):
                            k = k0 + dk
                            stg = stpool.tile([128, 1024], F32R,
                                              name=f"stg{cch}_{k}",
                                              tag="stg", bufs=3)
                            for s in range(4):
                                blk = cch * 4 + s
                                ps1 = ps1pool.tile([128, 256], F32,
                                                   name=f"ps1_{cch}_{k}_{s}",
                                                   tag="ps1", bufs=4)
                                dgs = []
                                for j in range(4):
                                    dg = dpool.tile([128, 128], F16,
                                                    name=f"dg{cch}_{k}_{s}_{j}",
                                                    tag=f"dg{j}", bufs=3)
                                    nc.vector.tensor_scalar(
                                        dg[:], maskh_s[:],
                                        a_s[j][:, blk, k: k + 1], None, OP.mult)
                                    dgs.append(dg)
                                cA = [gt[:, dk * 4 + s, joff[j]: joff[j] + 128]
                                      for j in range(4)]
                                cB = [gt[:, dk * 4 + s,
                                         joff[j] + 128: joff[j] + 256]
                                      for j in range(4)]
                                for j in range(4):
                                    nc.tensor.matmul(
                                        ps1[:, 0:128], cA[j], dgs[j][:],
                                        start=(j == 0), stop=(j == 3))
                                for j in range(4):
                                    nc.tensor.matmul(
                                        ps1[:, 128:256], cB[j], dgs[j][:],
                                        start=(j == 0), stop=(j == 3))
                                nc.scalar.copy(stg[:, s * 128:(s + 1) * 128],
                                               ps1[:, 0:128])
                                nc.vector.tensor_copy(
                                    stg[:, 512 + s * 128: 512 + (s + 1) * 128],
                                    ps1[:, 128:256])  # B-half on DVE
                            # pass-2: 4 groups, fp32r, N=512
                            for g in range(4):
                                half = g // 2
                                rb = 64 * (g % 2)
                                lhsT = wdefr_s[
                                    rb: rb + 64,
                                    (g * 9 + k) * 48:(g * 9 + k + 1) * 48]
                                nc.tensor.matmul(
                                    pos[g][0:48, :],
                                    lhsT,
                                    stg[rb: rb + 64, half * 512:(half + 1) * 512],
                                    start=(k == 0), stop=(k == KK - 1))
                    ob = obpool.tile([48, 4, 512], F32, name=f"ob{cch}",
                                     tag="ob", bufs=2)
                    cs = slice(cch * 512, (cch + 1) * 512)
                    for g in range(4):
                        nc.vector.tensor_copy(ob[:, g, :], pos[g][0:48, :])
                        nc.sync.dma_start(out_h[g * 48:(g + 1) * 48, cs],
                                          ob[:, g, :])

    nc.compile()
    return nc


def _host_inputs(x, w_off, b_off, w_def):
    """Per-core input dicts (8 cores = 4 batches x 2 halves)."""
    x = np.ascontiguousarray(x, np.float32)
    ky = np.repeat(np.arange(K), K).astype(np.float32)
    kx = np.tile(np.arange(K), K).astype(np.float32)

    # gather image: [b, 9216, 512] fp16, row r = [ch(y,x), ch(y+1,x)]
    xT = np.zeros((B, GDIM, GDIM, GCH), np.float16)
    xv = x.transpose(0, 2, 3, 1)
    for g in range(G):
        xT[:, PADG:PADG + H, PADG:PADG + W, 64 * g:64 * g + 48] = \
            xv[:, :, :, 48 * g:48 * (g + 1)]
    xT = xT.reshape(B, NROWS, GCH)
    xT2 = np.zeros((B, NROWS, 2 * GCH), np.float16)
    xT2[:, :, :GCH] = xT
    xT2[:, :NROWS - GDIM, GCH:] = xT[:, GDIM:]
    xT = xT2

    # conv image: pad-1, [b, 192, 66, 66]
    xc_full = np.zeros((B, C, 66, 66), np.float32)
    xc_full[:, :, 1:65, 1:65] = x

    wofft = np.zeros((96, 324), np.float32)
    for u in range(2):
        for k in range(KK):
            wofft[:, (u * 9 + k) * 18:(u * 9 + k + 1) * 18] = \
                w_off[:, u * 96:(u + 1) * 96, k // 3, k % 3].T
    wdeft = np.zeros((128, 1728), np.float32)
    for g in range(G):
        rb = 64 * (g % 2)
        for k in range(KK):
            wdeft[rb: rb + 48, (g * 9 + k) * 48:(g * 9 + k + 1) * 48] = \
                w_def[g * 48:(g + 1) * 48, :, k // 3, k % 3].T

    ident = np.eye(128, dtype=np.float32)
    maskh = np.eye(128, dtype=np.float16)
    selfold = np.zeros((128, 512), np.float32)
    for phi in range(8):
        for j in range(64):
            selfold[16 * phi + (j % 16), phi * 64 + j] = 1.0
    boff = np.asarray(b_off, np.float32).reshape(18, 1)

    p = np.arange(128)
    blkv = np.arange(16)
    baseX = ((p % 64)[:, None, None] - 1 + kx[None, None, :] + PADG
             + np.zeros((1, 16, 1))).astype(np.float32)

    in_maps = []
    for core in range(8):
        b, half = core // 2, core % 2
        h0 = half * HP
        hloc = h0 + blkv[None, :, None] * 2 + (p[:, None, None] // 64)
        baseY = (hloc - 1 + ky[None, None, :] + PADG).astype(np.float32)
        xc = np.zeros((2, 96, 34, 66), np.float32)
        win = xc_full[b, :, h0:h0 + 34, :]          # rows h0-1..h0+32 padded
        xc[0] = win[0:96]
        xc[1] = win[96:192]
        in_maps.append({
            "xT": xT[b], "xc": xc, "wofft": wofft, "boff": boff,
            "wdeft": wdeft, "ident": ident, "maskh": maskh,
            "selfold": selfold,
            "baseY": baseY, "baseX": np.ascontiguousarray(baseX),
        })
    return in_maps


def kernel(x, w_off, b_off, w_def):
    if "nc" not in _CACHE:
        _CACHE["nc"] = _build_nc()
    nc = _CACHE["nc"]
    in_maps = _host_inputs(np.asarray(x), np.asarray(w_off),
                           np.asarray(b_off), np.asarray(w_def))
    res = run_bass_kernel_spmd(nc, in_maps, core_ids=list(range(8)))
    out = np.zeros((B, C, H, W), np.float32)
    for core in range(8):
        b, half = core // 2, core % 2
        out[b, :, half * HP:(half + 1) * HP, :] = \
            res.results[core]["out"].reshape(C, HP, W)
    return out



# revision 22
# speedup vs baseline: 1.6571x; 1.1620x over previous
"""Deformable conv (nn_DeformConv) Trainium2 Bass kernel.

Sharding: 8 cores = 4 batches x 2 H-halves (spatial). Each core computes
out[b, :, h0:h0+32, :] for its half.

Per-core pipeline:
  1. offset conv (fp32r GEMM over shifted slices of the pad-1 image)
  2. PE-transpose offsets to [pixel-part, (blk, ch)] layout
  3. DVE index/alpha math (floor via int-cast, bilinear weights)
  4. PE "fold" of fp32 indices into the SWDGE idx wrap layout, cast int16
  5. dma_gather (fp16, overlapping x-pair elems) from the padded transposed
     image xT [9216, 256]
  6. pass-1: bilinear combine + transpose via diagonal matmuls into PSUM
  7. pass-2: grouped GEMM (fp32r) accumulating 9 taps into PSUM
"""

import dataclasses
import numpy as np
from contextlib import ExitStack

import concourse.bacc as bacc
import concourse.mybir as mybir
from concourse.tile import TileContext
from concourse.library_config import mlp
from concourse.bass_utils import run_bass_kernel_spmd

F16 = mybir.dt.float16
BF16 = mybir.dt.bfloat16
F32 = mybir.dt.float32
F32R = mybir.dt.float32r
I16 = mybir.dt.int16
I32 = mybir.dt.int32
OP = mybir.AluOpType

B, C, H, W = 4, 192, 64, 64
K, KK, G = 3, 9, 4
HP = 32                # output rows per core
PIX = HP * W           # 2048 pixels per core
NBLK = PIX // 128      # 16 pixel blocks of 128
NCH = 4                # 512-pixel chunks
PADG = 16              # gather-image pad on each side
GDIM = H + 2 * PADG    # 96
GCH = 256              # padded channel count in xT rows
NROWS = GDIM * GDIM    # 9216
WRAPM = (KK * PIX) // 16  # 1152 wrap columns (one idx per pixel*tap)

_CACHE = {}


def _to_bf16(a):
    import ml_dtypes
    return np.asarray(a, dtype=ml_dtypes.bfloat16)


def _replace_ap(ap, new_free_dims, extra_offset=0):
    return dataclasses.replace(
        ap, ap=[ap.ap[0]] + [list(d) for d in new_free_dims],
        offset=ap.offset + extra_offset,
    )


def _build_nc():
    # 2 SWDGE queues so consecutive gathers don't share a descriptor ring
    # (ring holds 64 data descs/engine -> at most 1024 idxs per gather).
    nc = bacc.Bacc("TRN2", target_bir_lowering=False, num_swdge_queues=2)

    # xT rows are y-pair concatenated: row r = [ch(y,x), ch(y+1,x)] so one
    # gather descriptor (2 consecutive rows) covers all 4 bilinear corners.
    xT_h = nc.dram_tensor("xT", [NROWS, 2 * GCH], F16, kind="ExternalInput")
    xc_h = nc.dram_tensor("xc", [2, 96, 34, 66], F32, kind="ExternalInput")
    wofft_h = nc.dram_tensor("wofft", [96, 324], F32, kind="ExternalInput")
    boff_h = nc.dram_tensor("boff", [18, 1], F32, kind="ExternalInput")
    wdeft_h = nc.dram_tensor("wdeft", [128, 1728], F32, kind="ExternalInput")
    ident_h = nc.dram_tensor("ident", [128, 128], F32, kind="ExternalInput")
    maskh_h = nc.dram_tensor("maskh", [128, 128], F16, kind="ExternalInput")
    selfold_h = nc.dram_tensor("selfold", [128, 512], F32, kind="ExternalInput")
    baseY_h = nc.dram_tensor("baseY", [128, 16, 9], F32, kind="ExternalInput")
    baseX_h = nc.dram_tensor("baseX", [128, 16, 9], F32, kind="ExternalInput")
    out_h = nc.dram_tensor("out", [192, PIX], F32, kind="ExternalOutput")

    with TileContext(nc) as tc:
        nc.gpsimd.load_library(mlp)
        with ExitStack() as ctx:
            cpool = ctx.enter_context(tc.tile_pool(name="const", bufs=1))

            maskh_s = cpool.tile([128, 128], F16)
            nc.sync.dma_start(maskh_s[:], maskh_h[:])
            wdefr_s = cpool.tile([128, 1728], F32R)

            # persistent across main loop: packed bilinear weights
            # a4_s[p, blk, k, j] fp16
            a4_s = cpool.tile([128, 16, 9, 4], F16, name="alpha4")
            wrap_s = cpool.tile([128, WRAPM], I16)
            nc.vector.memset(wrap_s[:], 0)

            # ---------- prologue ----------
            with ExitStack() as pctx:
                ppool = pctx.enter_context(tc.tile_pool(name="prol", bufs=1))
                ppsum_off = pctx.enter_context(
                    tc.tile_pool(name="ppso", bufs=2, space="PSUM"))
                ppsum_t = pctx.enter_context(
                    tc.tile_pool(name="ppst", bufs=2, space="PSUM"))
                ppsum_f = pctx.enter_context(
                    tc.tile_pool(name="ppsf", bufs=2, space="PSUM"))

                xc_s = ppool.tile([96, 2, 34, 66], F32)
                nc.sync.dma_start(xc_s[:, 0], xc_h[0])
                nc.sync.dma_start(xc_s[:, 1], xc_h[1])
                wofft_s = ppool.tile([96, 324], F32)
                nc.sync.dma_start(wofft_s[:], wofft_h[:])
                boff_s = ppool.tile([18, 1], F32)
                nc.sync.dma_start(boff_s[:], boff_h[:])
                wdeft_s = ppool.tile([128, 1728], F32)
                nc.sync.dma_start(wdeft_s[:], wdeft_h[:])
                ident_s = ppool.tile([128, 128], F32)
                nc.sync.dma_start(ident_s[:], ident_h[:])
                selfold_s = ppool.tile([128, 512], F32)
                nc.sync.dma_start(selfold_s[:], selfold_h[:])
                baseY_s = ppool.tile([128, 16, 9], F32)
                nc.sync.dma_start(baseY_s[:], baseY_h[:])
                baseX_s = ppool.tile([128, 16, 9], F32)
                nc.sync.dma_start(baseX_s[:], baseX_h[:])

                # fp32r-rounded copies (fp32r matmul operands must be
                # produced by a rounding instruction)
                xcr_s = ppool.tile([96, 2, 34, 66], F32R)
                nc.vector.tensor_copy(xcr_s[:], xc_s[:])
                woffr_s = ppool.tile([96, 324], F32R)
                nc.vector.tensor_copy(woffr_s[:], wofft_s[:])
                nc.vector.tensor_copy(wdefr_s[:], wdeft_s[:])

                # Stage A: offset conv -> offs_s [18, 2048]
                offs_s = ppool.tile([18, PIX], F32)
                for nch in range(NCH):
                    pco = ppsum_off.tile([18, 512], F32, name=f"pco{nch}",
                                         tag="pco", bufs=2)
                    first = True
                    for u in range(2):
                        for k in range(KK):
                            ky, kx = k // 3, k % 3
                            rhs = xcr_s[:, u, nch * 8 + ky: nch * 8 + ky + 8,
                                        kx: kx + 64]
                            lhsT = woffr_s[:, (u * 9 + k) * 18:(u * 9 + k + 1) * 18]
                            nc.tensor.matmul(
                                pco[:], lhsT, rhs,
                                start=first, stop=(u == 1 and k == KK - 1))
                            first = False
                    nc.vector.tensor_scalar(
                        offs_s[:, nch * 512:(nch + 1) * 512], pco[:],
                        boff_s[:, 0:1], None, OP.add)

                # Stage B: transpose -> offT_s [128, (blk, 18)]
                offT_s = ppool.tile([128, 16 * 18], F32)
                for blk in range(NBLK):
                    pt = ppsum_t.tile([128, 18], F32, name=f"pt{blk}",
                                      tag="pt", bufs=2)
                    nc.tensor.transpose(
                        pt[:], offs_s[:, blk * 128:(blk + 1) * 128],
                        ident_s[0:18, 0:18])
                    nc.scalar.copy(offT_s[:, blk * 18:(blk + 1) * 18], pt[:])

                # Stage C: index/alpha math on [128, (16, 9)]
                base = offT_s[:, :]
                dy = _replace_ap(base, [[18, 16], [2, 9]])
                dx = _replace_ap(base, [[18, 16], [2, 9]], extra_offset=1)

                def floorfrac(src_off, base_s, clamp_hi, nm):
                    v = ppool.tile([128, 16, 9], F32, name=f"v{nm}")
                    nc.vector.tensor_tensor(v[:], src_off, base_s[:], OP.add)
                    nc.vector.tensor_scalar(v[:], v[:], 1.5, clamp_hi,
                                            OP.max, OP.min)
                    vi = ppool.tile([128, 16, 9], I32, name=f"vi{nm}")
                    nc.vector.tensor_copy(vi[:], v[:])
                    vr = ppool.tile([128, 16, 9], F32, name=f"vr{nm}")
                    nc.vector.tensor_copy(vr[:], vi[:])
                    corr = ppool.tile([128, 16, 9], F32, name=f"corr{nm}")
                    nc.vector.tensor_tensor(corr[:], vr[:], v[:], OP.is_gt)
                    v0 = ppool.tile([128, 16, 9], F32, name=f"v0{nm}")
                    nc.vector.tensor_tensor(v0[:], vr[:], corr[:], OP.subtract)
                    fr = ppool.tile([128, 16, 9], F32, name=f"fr{nm}")
                    nc.vector.tensor_tensor(fr[:], v[:], v0[:], OP.subtract)
                    return v0, fr

                y0p, fy = floorfrac(dy, baseY_s, 94.5, "y")
                x0p, fx = floorfrac(dx, baseX_s, 93.5, "x")
                gy = ppool.tile([128, 16, 9], F32)
                nc.vector.tensor_scalar(gy[:], fy[:], -1.0, 1.0, OP.mult, OP.add)
                gx = ppool.tile([128, 16, 9], F32)
                nc.vector.tensor_scalar(gx[:], fx[:], -1.0, 1.0, OP.mult, OP.add)
                for j, (ta, tb) in enumerate([(gy, gx), (gy, fx), (fy, gx), (fy, fx)]):
                    dst = _replace_ap(a4_s[:, :, :, :], [[36, 16], [4, 9]],
                                      extra_offset=j)
                    nc.vector.tensor_tensor(dst, ta[:], tb[:], OP.mult)

                # idxf [128, (c4, k9, s4)] fp32 — one idx per (pixel, tap)
                # (row of the y-pair image; the 4-corner elem covers the rest)
                idxf = ppool.tile([128, 144], F32)
                for c4 in range(4):
                    srcd = [[1, 9], [9, 4]]
                    y0_ap = _replace_ap(y0p[:, :], srcd, extra_offset=c4 * 36)
                    x0_ap = _replace_ap(x0p[:, :], srcd, extra_offset=c4 * 36)
                    dstd = [[4, 9], [1, 4]]
                    idxA = _replace_ap(idxf[:, :], dstd, extra_offset=c4 * 36)
                    nc.vector.scalar_tensor_tensor(
                        idxA, y0_ap, float(GDIM), x0_ap, OP.mult, OP.add)

                # Stage D: fold into wrap layout, cast int16.
                # wrap[w, (cch*9+k)*32 + 8*q + e] = idx(pix=cch*512+q*128
                # +e*16+w, tap k) = idxf[e*16+w, cch*36+4k+q], dup at w+16.
                for cch in range(4):
                    pf = ppsum_f.tile([64, 8, 36], F32, name=f"pf{cch}",
                                      tag="pf", bufs=2)
                    for e in range(8):
                        nc.tensor.matmul(
                            pf[:, e, :],
                            selfold_s[:, e * 64: e * 64 + 64],
                            idxf[:, cch * 36:(cch + 1) * 36],
                            start=True, stop=True)
                    for k in range(KK):
                        dst = _replace_ap(wrap_s[0:64, :], [[8, 4], [1, 8]],
                                          extra_offset=(cch * 9 + k) * 32)
                        src = _replace_ap(pf[:, :, :], [[1, 4], [36, 8]],
                                          extra_offset=4 * k)
                        nc.scalar.copy(dst, src)

            # ---------- main loop ----------
            with ExitStack() as mctx:
                gpool = mctx.enter_context(tc.tile_pool(name="gat", bufs=2))
                dpool = mctx.enter_context(tc.tile_pool(name="diag", bufs=2))
                stpool = mctx.enter_context(tc.tile_pool(name="stg", bufs=2))
                obpool = mctx.enter_context(tc.tile_pool(name="ob", bufs=2))
                ps1pool = mctx.enter_context(
                    tc.tile_pool(name="ps1", bufs=2, space="PSUM"))
                popool = mctx.enter_context(
                    tc.tile_pool(name="po", bufs=2, space="PSUM"))

                pair_ap = dataclasses.replace(
                    xT_h[:, :], ap=[[2 * GCH, NROWS - 1], [1, 4 * GCH]])
                # corner j -> offset in the 1024-elem 4-corner gather row:
                # [0:256]=(y0,x0) [256:512]=(y1,x0) [512:768]=(y0,x1)
                # [768:1024]=(y1,x1); alpha order j = 00,01,10,11
                joff = [0, 512, 256, 768]

                for cch in range(NCH):
                    pos = [popool.tile([64, 512], F32, name=f"po{cch}_{g}",
                                       tag=f"po{g}", bufs=1) for g in range(4)]
                    for kb, taps in enumerate([2, 2, 2, 2, 1]):
                        k0 = 2 * kb
                        gt = gpool.tile([128, 4 * taps, 4 * GCH], F16,
                                        name=f"gt{cch}_{kb}",
                                        tag=f"gt{taps}", bufs=2)
                        callc = (cch * 9 + k0) * 32
                        nc.gpsimd.dma_gather(
                            gt[:], pair_ap,
                            wrap_s[:, callc: callc + 32 * taps],
                            512 * taps, 512 * taps, 4 * GCH,
                            elem_step=2 * GCH,
                            queue_num=(cch * 5 + kb) % 2)
                        for dk in range(taps):
                            k = k0 + dk
                            # stg layout [128, (s4, half2, 128)]
                            stg = stpool.tile([128, 4, 2, 128], F32R,
                                              name=f"stg{cch}_{k}",
                                              tag="stg", bufs=3)
                            for s in range(4):
                                blk = cch * 4 + s
                                ps1 = ps1pool.tile([128, 256], F32,
                                                   name=f"ps1_{cch}_{k}_{s}",
                                                   tag="ps1", bufs=4)
                                # all 4 alpha-diagonals in one DVE op:
                                # dg4[p, j, q] = maskh[p, q] * a4[p, blk, k, j]
                                dg4 = dpool.tile([128, 4, 128], F16,
                                                 name=f"dg{cch}_{k}_{s}",
                                                 tag="dg4", bufs=3)
                                mrep = _replace_ap(maskh_s[:, :],
                                                   [[0, 4], [1, 128]])
                                arep = _replace_ap(a4_s[:, :, :, :],
                                                   [[1, 4], [0, 128]],
                                                   extra_offset=blk * 36 + k * 4)
                                nc.vector.tensor_tensor(dg4[:], mrep, arep,
                                                        OP.mult)
                                cA = [gt[:, dk * 4 + s, joff[j]: joff[j] + 128]
                                      for j in range(4)]
                                cB = [gt[:, dk * 4 + s,
                                         joff[j] + 128: joff[j] + 256]
                                      for j in range(4)]
                                for j in range(4):
                                    nc.tensor.matmul(
                                        ps1[:, 0:128], cA[j], dg4[:, j, :],
                                        start=(j == 0), stop=(j == 3))
                                for j in range(4):
                                    nc.tensor.matmul(
                                        ps1[:, 128:256], cB[j], dg4[:, j, :],
                                        start=(j == 0), stop=(j == 3))
                                nc.scalar.copy(stg[:, s, :, :], ps1[:])
                            # pass-2: 4 groups, fp32r, N=512, strided rhs
                            for g in range(4):
                                half = g // 2
                                rb = 64 * (g % 2)
                                lhsT = wdefr_s[
                                    rb: rb + 64,
                                    (g * 9 + k) * 48:(g * 9 + k + 1) * 48]
                                rhs = _replace_ap(
                                    stg[rb: rb + 64, :, :, :],
                                    [[256, 4], [1, 128]],
                                    extra_offset=half * 128)
                                nc.tensor.matmul(
                                    pos[g][0:48, :], lhsT, rhs,
                                    start=(k == 0), stop=(k == KK - 1))
                    ob = obpool.tile([48, 4, 512], F32, name=f"ob{cch}",
                                     tag="ob", bufs=2)
                    cs = slice(cch * 512, (cch + 1) * 512)
                    for g in range(4):
                        nc.vector.tensor_copy(ob[:, g, :], pos[g][0:48, :])
                        nc.sync.dma_start(out_h[g * 48:(g + 1) * 48, cs],
                                          ob[:, g, :])

    nc.compile()
    return nc


def _host_inputs(x, w_off, b_off, w_def):
    """Per-core input dicts (8 cores = 4 batches x 2 halves)."""
    x = np.ascontiguousarray(x, np.float32)
    ky = np.repeat(np.arange(K), K).astype(np.float32)
    kx = np.tile(np.arange(K), K).astype(np.float32)

    # gather image: [b, 9216, 512] fp16, row r = [ch(y,x), ch(y+1,x)]
    xT = np.zeros((B, GDIM, GDIM, GCH), np.float16)
    xv = x.transpose(0, 2, 3, 1)
    for g in range(G):
        xT[:, PADG:PADG + H, PADG:PADG + W, 64 * g:64 * g + 48] = \
            xv[:, :, :, 48 * g:48 * (g + 1)]
    xT = xT.reshape(B, NROWS, GCH)
    xT2 = np.zeros((B, NROWS, 2 * GCH), np.float16)
    xT2[:, :, :GCH] = xT
    xT2[:, :NROWS - GDIM, GCH:] = xT[:, GDIM:]
    xT = xT2

    # conv image: pad-1, [b, 192, 66, 66]
    xc_full = np.zeros((B, C, 66, 66), np.float32)
    xc_full[:, :, 1:65, 1:65] = x

    wofft = np.zeros((96, 324), np.float32)
    for u in range(2):
        for k in range(KK):
            wofft[:, (u * 9 + k) * 18:(u * 9 + k + 1) * 18] = \
                w_off[:, u * 96:(u + 1) * 96, k // 3, k % 3].T
    wdeft = np.zeros((128, 1728), np.float32)
    for g in range(G):
        rb = 64 * (g % 2)
        for k in range(KK):
            wdeft[rb: rb + 48, (g * 9 + k) * 48:(g * 9 + k + 1) * 48] = \
                w_def[g * 48:(g + 1) * 48, :, k // 3, k % 3].T

    ident = np.eye(128, dtype=np.float32)
    maskh = np.eye(128, dtype=np.float16)
    selfold = np.zeros((128, 512), np.float32)
    for phi in range(8):
        for j in range(64):
            selfold[16 * phi + (j % 16), phi * 64 + j] = 1.0
    boff = np.asarray(b_off, np.float32).reshape(18, 1)

    p = np.arange(128)
    blkv = np.arange(16)
    baseX = ((p % 64)[:, None, None] - 1 + kx[None, None, :] + PADG
             + np.zeros((1, 16, 1))).astype(np.float32)

    in_maps = []
    for core in range(8):
        b, half = core // 2, core % 2
        h0 = half * HP
        hloc = h0 + blkv[None, :, None] * 2 + (p[:, None, None] // 64)
        baseY = (hloc - 1 + ky[None, None, :] + PADG).astype(np.float32)
        xc = np.zeros((2, 96, 34, 66), np.float32)
        win = xc_full[b, :, h0:h0 + 34, :]          # rows h0-1..h0+32 padded
        xc[0] = win[0:96]
        xc[1] = win[96:192]
        in_maps.append({
            "xT": xT[b], "xc": xc, "wofft": wofft, "boff": boff,
            "wdeft": wdeft, "ident": ident, "maskh": maskh,
            "selfold": selfold,
            "baseY": baseY, "baseX": np.ascontiguousarray(baseX),
        })
    return in_maps


def kernel(x, w_off, b_off, w_def):
    if "nc" not in _CACHE:
        _CACHE["nc"] = _build_nc()
    nc = _CACHE["nc"]
    in_maps = _host_inputs(np.asarray(x), np.asarray(w_off),
                           np.asarray(b_off), np.asarray(w_def))
    res = run_bass_kernel_spmd(nc, in_maps, core_ids=list(range(8)))
    out = np.zeros((B, C, H, W), np.float32)
    for core in range(8):
        b, half = core // 2, core % 2
        out[b, :, half * HP:(half + 1) * HP, :] = \
            res.results[core]["out"].reshape(C, HP, W)
    return out



# revision 24
# speedup vs baseline: 1.8600x; 1.1224x over previous
"""Deformable conv (nn_DeformConv) Trainium2 Bass kernel.

Sharding: 8 cores = 4 batches x 2 H-halves (spatial). Each core computes
out[b, :, h0:h0+32, :] for its half.

Per-core pipeline:
  1. offset conv (fp32r GEMM over shifted slices of the pad-1 image)
  2. PE-transpose offsets to [pixel-part, (blk, ch)] layout
  3. DVE index/alpha math (floor via int-cast, bilinear weights)
  4. PE "fold" of fp32 indices into the SWDGE idx wrap layout, cast int16
  5. dma_gather (fp16, overlapping x-pair elems) from the padded transposed
     image xT [9216, 256]
  6. pass-1: bilinear combine + transpose via diagonal matmuls into PSUM
  7. pass-2: grouped GEMM (fp32r) accumulating 9 taps into PSUM
"""

import dataclasses
import numpy as np
from contextlib import ExitStack

import concourse.bacc as bacc
import concourse.mybir as mybir
from concourse.tile import TileContext
from concourse.library_config import mlp
from concourse.bass_utils import run_bass_kernel_spmd

F16 = mybir.dt.float16
BF16 = mybir.dt.bfloat16
F32 = mybir.dt.float32
F32R = mybir.dt.float32r
I16 = mybir.dt.int16
I32 = mybir.dt.int32
OP = mybir.AluOpType

B, C, H, W = 4, 192, 64, 64
K, KK, G = 3, 9, 4
HP = 32                # output rows per core
PIX = HP * W           # 2048 pixels per core
NBLK = PIX // 128      # 16 pixel blocks of 128
NCH = 4                # 512-pixel chunks
PADG = 16              # gather-image pad on each side
GDIM = H + 2 * PADG    # 96
GCH = 256              # padded channel count in xT rows
NROWS = GDIM * GDIM    # 9216
WRAPM = (KK * PIX) // 16  # 1152 wrap columns (one idx per pixel*tap)

_CACHE = {}


def _to_bf16(a):
    import ml_dtypes
    return np.asarray(a, dtype=ml_dtypes.bfloat16)


def _replace_ap(ap, new_free_dims, extra_offset=0):
    return dataclasses.replace(
        ap, ap=[ap.ap[0]] + [list(d) for d in new_free_dims],
        offset=ap.offset + extra_offset,
    )


def _build_nc():
    # 2 SWDGE queues so consecutive gathers don't share a descriptor ring
    # (ring holds 64 data descs/engine -> at most 1024 idxs per gather).
    nc = bacc.Bacc("TRN2", target_bir_lowering=False, num_swdge_queues=2)

    # xT rows are y-pair concatenated: row r = [ch(y,x), ch(y+1,x)] so one
    # gather descriptor (2 consecutive rows) covers all 4 bilinear corners.
    xT_h = nc.dram_tensor("xT", [NROWS, 2 * GCH], F16, kind="ExternalInput")
    xc_h = nc.dram_tensor("xc", [2, 96, 34, 66], F32, kind="ExternalInput")
    wofft_h = nc.dram_tensor("wofft", [96, 324], F32, kind="ExternalInput")
    boff_h = nc.dram_tensor("boff", [18, 1], F32, kind="ExternalInput")
    wdeft_h = nc.dram_tensor("wdeft", [128, 1728], F32, kind="ExternalInput")
    ident_h = nc.dram_tensor("ident", [128, 128], F32, kind="ExternalInput")
    maskh_h = nc.dram_tensor("maskh", [128, 128], F16, kind="ExternalInput")
    selfold_h = nc.dram_tensor("selfold", [128, 512], F32, kind="ExternalInput")
    baseY_h = nc.dram_tensor("baseY", [128, 16, 9], F32, kind="ExternalInput")
    baseX_h = nc.dram_tensor("baseX", [128, 16, 9], F32, kind="ExternalInput")
    out_h = nc.dram_tensor("out", [192, PIX], F32, kind="ExternalOutput")

    with TileContext(nc) as tc:
        nc.gpsimd.load_library(mlp)
        with ExitStack() as ctx:
            cpool = ctx.enter_context(tc.tile_pool(name="const", bufs=1))

            maskh_s = cpool.tile([128, 128], F16)
            nc.sync.dma_start(maskh_s[:], maskh_h[:])
            wdefr_s = cpool.tile([128, 1728], F32R)

            # persistent across main loop: packed bilinear weights
            # a4_s[p, blk, k, j] fp16
            a4_s = cpool.tile([128, 16, 9, 4], F16, name="alpha4")
            wrap_s = cpool.tile([128, WRAPM], I16)
            nc.vector.memset(wrap_s[:], 0)

            # warm the dma_gather ucode on both queues while the prologue
            # computes (first invocation pays ~20us icache cost)
            warm_ap = dataclasses.replace(
                xT_h[:, :], ap=[[2 * GCH, NROWS - 1], [1, 128]])
            warm_s = cpool.tile([128, 1, 128], F16, name="warm")
            for q in range(2):
                nc.gpsimd.dma_gather(
                    warm_s[:], warm_ap, wrap_s[:, 0:8], 128, 128, 128,
                    elem_step=2 * GCH, queue_num=q)

            # ---------- prologue ----------
            with ExitStack() as pctx:
                ppool = pctx.enter_context(tc.tile_pool(name="prol", bufs=1))
                ppsum_off = pctx.enter_context(
                    tc.tile_pool(name="ppso", bufs=2, space="PSUM"))
                ppsum_t = pctx.enter_context(
                    tc.tile_pool(name="ppst", bufs=2, space="PSUM"))
                ppsum_f = pctx.enter_context(
                    tc.tile_pool(name="ppsf", bufs=2, space="PSUM"))

                xc_s = ppool.tile([96, 2, 34, 66], F32)
                nc.sync.dma_start(xc_s[:, 0], xc_h[0])
                nc.sync.dma_start(xc_s[:, 1], xc_h[1])
                wofft_s = ppool.tile([96, 324], F32)
                nc.sync.dma_start(wofft_s[:], wofft_h[:])
                boff_s = ppool.tile([18, 1], F32)
                nc.sync.dma_start(boff_s[:], boff_h[:])
                wdeft_s = ppool.tile([128, 1728], F32)
                nc.sync.dma_start(wdeft_s[:], wdeft_h[:])
                ident_s = ppool.tile([128, 128], F32)
                nc.sync.dma_start(ident_s[:], ident_h[:])
                selfold_s = ppool.tile([128, 512], F32)
                nc.sync.dma_start(selfold_s[:], selfold_h[:])
                baseY_s = ppool.tile([128, 16, 9], F32)
                nc.sync.dma_start(baseY_s[:], baseY_h[:])
                baseX_s = ppool.tile([128, 16, 9], F32)
                nc.sync.dma_start(baseX_s[:], baseX_h[:])

                # fp32r-rounded copies (fp32r matmul operands must be
                # produced by a rounding instruction)
                xcr_s = ppool.tile([96, 2, 34, 66], F32R)
                nc.vector.tensor_copy(xcr_s[:], xc_s[:])
                woffr_s = ppool.tile([96, 324], F32R)
                nc.vector.tensor_copy(woffr_s[:], wofft_s[:])
                nc.vector.tensor_copy(wdefr_s[:], wdeft_s[:])

                # Stages A-D pipelined per 512-pixel chunk c4, so the
                # first gathers start ~1/4 into the prologue.
                offs_s = ppool.tile([18, PIX], F32)
                offT_s = ppool.tile([128, 16 * 18], F32)
                idxf = ppool.tile([128, 144], F32)

                def floorfrac(c4, src_off, base_sl, clamp_hi, nm):
                    v = ppool.tile([128, 4, 9], F32, name=f"v{nm}{c4}")
                    nc.vector.tensor_tensor(v[:], src_off, base_sl, OP.add)
                    nc.vector.tensor_scalar(v[:], v[:], 1.5, clamp_hi,
                                            OP.max, OP.min)
                    vi = ppool.tile([128, 4, 9], I32, name=f"vi{nm}{c4}")
                    nc.vector.tensor_copy(vi[:], v[:])
                    vr = ppool.tile([128, 4, 9], F32, name=f"vr{nm}{c4}")
                    nc.vector.tensor_copy(vr[:], vi[:])
                    corr = ppool.tile([128, 4, 9], F32, name=f"corr{nm}{c4}")
                    nc.vector.tensor_tensor(corr[:], vr[:], v[:], OP.is_gt)
                    v0 = ppool.tile([128, 4, 9], F32, name=f"v0{nm}{c4}")
                    nc.vector.tensor_tensor(v0[:], vr[:], corr[:], OP.subtract)
                    fr = ppool.tile([128, 4, 9], F32, name=f"fr{nm}{c4}")
                    nc.vector.tensor_tensor(fr[:], v[:], v0[:], OP.subtract)
                    return v0, fr

                for c4 in range(4):
                    # Stage A: offset conv for this chunk
                    pco = ppsum_off.tile([18, 512], F32, name=f"pco{c4}",
                                         tag="pco", bufs=2)
                    first = True
                    for u in range(2):
                        for k in range(KK):
                            ky, kx = k // 3, k % 3
                            rhs = xcr_s[:, u, c4 * 8 + ky: c4 * 8 + ky + 8,
                                        kx: kx + 64]
                            lhsT = woffr_s[:, (u * 9 + k) * 18:(u * 9 + k + 1) * 18]
                            nc.tensor.matmul(
                                pco[:], lhsT, rhs,
                                start=first, stop=(u == 1 and k == KK - 1))
                            first = False
                    nc.vector.tensor_scalar(
                        offs_s[:, c4 * 512:(c4 + 1) * 512], pco[:],
                        boff_s[:, 0:1], None, OP.add)

                    # Stage B: transpose 4 blocks -> offT_s
                    for blk in range(4 * c4, 4 * c4 + 4):
                        pt = ppsum_t.tile([128, 18], F32, name=f"pt{blk}",
                                          tag="pt", bufs=2)
                        nc.tensor.transpose(
                            pt[:], offs_s[:, blk * 128:(blk + 1) * 128],
                            ident_s[0:18, 0:18])
                        nc.scalar.copy(offT_s[:, blk * 18:(blk + 1) * 18], pt[:])

                    # Stage C: index/alpha math on [128, (4, 9)]
                    dy = _replace_ap(offT_s[:, :], [[18, 4], [2, 9]],
                                     extra_offset=c4 * 72)
                    dx = _replace_ap(offT_s[:, :], [[18, 4], [2, 9]],
                                     extra_offset=c4 * 72 + 1)
                    y0p, fy = floorfrac(c4, dy, baseY_s[:, 4 * c4: 4 * c4 + 4, :],
                                        94.5, "y")
                    x0p, fx = floorfrac(c4, dx, baseX_s[:, 4 * c4: 4 * c4 + 4, :],
                                        93.5, "x")
                    gy = ppool.tile([128, 4, 9], F32, name=f"gy{c4}")
                    nc.vector.tensor_scalar(gy[:], fy[:], -1.0, 1.0,
                                            OP.mult, OP.add)
                    gx = ppool.tile([128, 4, 9], F32, name=f"gx{c4}")
                    nc.vector.tensor_scalar(gx[:], fx[:], -1.0, 1.0,
                                            OP.mult, OP.add)
                    for j, (ta, tb) in enumerate(
                            [(gy, gx), (gy, fx), (fy, gx), (fy, fx)]):
                        dst = _replace_ap(a4_s[:, :, :, :], [[36, 4], [4, 9]],
                                          extra_offset=c4 * 144 + j)
                        nc.vector.tensor_tensor(dst, ta[:], tb[:], OP.mult)

                    srcd = [[1, 9], [9, 4]]
                    y0_ap = _replace_ap(y0p[:, :], srcd)
                    x0_ap = _replace_ap(x0p[:, :], srcd)
                    dstd = [[4, 9], [1, 4]]
                    idxA = _replace_ap(idxf[:, :], dstd, extra_offset=c4 * 36)
                    nc.vector.scalar_tensor_tensor(
                        idxA, y0_ap, float(GDIM), x0_ap, OP.mult, OP.add)

                    # Stage D: fold into wrap layout, cast int16.
                    # wrap[w, (c4*9+k)*32 + 8*q + e] = idxf[e*16+w,
                    # c4*36+4k+q], replicated across 64 partitions.
                    pf = ppsum_f.tile([64, 8, 36], F32, name=f"pf{c4}",
                                      tag="pf", bufs=2)
                    for e in range(8):
                        nc.tensor.matmul(
                            pf[:, e, :],
                            selfold_s[:, e * 64: e * 64 + 64],
                            idxf[:, c4 * 36:(c4 + 1) * 36],
                            start=True, stop=True)
                    for k in range(KK):
                        dst = _replace_ap(wrap_s[0:64, :], [[8, 4], [1, 8]],
                                          extra_offset=(c4 * 9 + k) * 32)
                        srcp = _replace_ap(pf[:, :, :], [[1, 4], [36, 8]],
                                           extra_offset=4 * k)
                        nc.scalar.copy(dst, srcp)

            # ---------- main loop ----------
            with ExitStack() as mctx:
                gpool = mctx.enter_context(tc.tile_pool(name="gat", bufs=2))
                dpool = mctx.enter_context(tc.tile_pool(name="diag", bufs=2))
                stpool = mctx.enter_context(tc.tile_pool(name="stg", bufs=2))
                obpool = mctx.enter_context(tc.tile_pool(name="ob", bufs=2))
                ps1pool = mctx.enter_context(
                    tc.tile_pool(name="ps1", bufs=2, space="PSUM"))
                popool = mctx.enter_context(
                    tc.tile_pool(name="po", bufs=2, space="PSUM"))

                pair_ap = dataclasses.replace(
                    xT_h[:, :], ap=[[2 * GCH, NROWS - 1], [1, 4 * GCH]])
                # corner j -> offset in the 1024-elem 4-corner gather row:
                # [0:256]=(y0,x0) [256:512]=(y1,x0) [512:768]=(y0,x1)
                # [768:1024]=(y1,x1); alpha order j = 00,01,10,11
                joff = [0, 512, 256, 768]

                for cch in range(NCH):
                    pos = [popool.tile([64, 512], F32, name=f"po{cch}_{g}",
                                       tag=f"po{g}", bufs=1) for g in range(4)]
                    for kb, taps in enumerate([2, 2, 2, 2, 1]):
                        k0 = 2 * kb
                        gt = gpool.tile([128, 4 * taps, 4 * GCH], F16,
                                        name=f"gt{cch}_{kb}",
                                        tag=f"gt{taps}", bufs=3)
                        callc = (cch * 9 + k0) * 32
                        nc.gpsimd.dma_gather(
                            gt[:], pair_ap,
                            wrap_s[:, callc: callc + 32 * taps],
                            512 * taps, 512 * taps, 4 * GCH,
                            elem_step=2 * GCH,
                            queue_num=(cch * 5 + kb) % 2)
                        for dk in range(taps):
                            k = k0 + dk
                            # stg layout [128, (s4, half2, 128)]
                            stg = stpool.tile([128, 4, 2, 128], F32R,
                                              name=f"stg{cch}_{k}",
                                              tag="stg", bufs=4)
                            for s in range(4):
                                blk = cch * 4 + s
                                ps1 = ps1pool.tile([128, 256], F32,
                                                   name=f"ps1_{cch}_{k}_{s}",
                                                   tag="ps1", bufs=4)
                                # all 4 alpha-diagonals in one DVE op:
                                # dg4[p, j, q] = maskh[p, q] * a4[p, blk, k, j]
                                dg4 = dpool.tile([128, 4, 128], F16,
                                                 name=f"dg{cch}_{k}_{s}",
                                                 tag="dg4", bufs=3)
                                mrep = _replace_ap(maskh_s[:, :],
                                                   [[0, 4], [1, 128]])
                                arep = _replace_ap(a4_s[:, :, :, :],
                                                   [[1, 4], [0, 128]],
                                                   extra_offset=blk * 36 + k * 4)
                                nc.vector.tensor_tensor(dg4[:], mrep, arep,
                                                        OP.mult)
                                cA = [gt[:, dk * 4 + s, joff[j]: joff[j] + 128]
                                      for j in range(4)]
                                cB = [gt[:, dk * 4 + s,
                                         joff[j] + 128: joff[j] + 256]
                                      for j in range(4)]
                                for j in range(4):
                                    nc.tensor.matmul(
                                        ps1[:, 0:128], cA[j], dg4[:, j, :],
                                        start=(j == 0), stop=(j == 3))
                                for j in range(4):
                                    nc.tensor.matmul(
                                        ps1[:, 128:256], cB[j], dg4[:, j, :],
                                        start=(j == 0), stop=(j == 3))
                                nc.scalar.copy(stg[:, s, :, :], ps1[:])
                            # pass-2: 4 groups, fp32r, N=512, strided rhs
                            for g in range(4):
                                half = g // 2
                                rb = 64 * (g % 2)
                                lhsT = wdefr_s[
                                    rb: rb + 64,
                                    (g * 9 + k) * 48:(g * 9 + k + 1) * 48]
                                rhs = _replace_ap(
                                    stg[rb: rb + 64, :, :, :],
                                    [[256, 4], [1, 128]],
                                    extra_offset=half * 128)
                                nc.tensor.matmul(
                                    pos[g][0:48, :], lhsT, rhs,
                                    start=(k == 0), stop=(k == KK - 1))
                    ob = obpool.tile([48, 4, 512], F32, name=f"ob{cch}",
                                     tag="ob", bufs=2)
                    cs = slice(cch * 512, (cch + 1) * 512)
                    for g in range(4):
                        nc.vector.tensor_copy(ob[:, g, :], pos[g][0:48, :])
                        nc.sync.dma_start(out_h[g * 48:(g + 1) * 48, cs],
                                          ob[:, g, :])

    nc.compile()
    return nc


def _host_inputs(x, w_off, b_off, w_def):
    """Per-core input dicts (8 cores = 4 batches x 2 halves)."""
    x = np.ascontiguousarray(x, np.float32)
    ky = np.repeat(np.arange(K), K).astype(np.float32)
    kx = np.tile(np.arange(K), K).astype(np.float32)

    # gather image: [b, 9216, 512] fp16, row r = [ch(y,x), ch(y+1,x)]
    xT = np.zeros((B, GDIM, GDIM, GCH), np.float16)
    xv = x.transpose(0, 2, 3, 1)
    for g in range(G):
        xT[:, PADG:PADG + H, PADG:PADG + W, 64 * g:64 * g + 48] = \
            xv[:, :, :, 48 * g:48 * (g + 1)]
    xT = xT.reshape(B, NROWS, GCH)
    xT2 = np.zeros((B, NROWS, 2 * GCH), np.float16)
    xT2[:, :, :GCH] = xT
    xT2[:, :NROWS - GDIM, GCH:] = xT[:, GDIM:]
    xT = xT2

    # conv image: pad-1, [b, 192, 66, 66]
    xc_full = np.zeros((B, C, 66, 66), np.float32)
    xc_full[:, :, 1:65, 1:65] = x

    wofft = np.zeros((96, 324), np.float32)
    for u in range(2):
        for k in range(KK):
            wofft[:, (u * 9 + k) * 18:(u * 9 + k + 1) * 18] = \
                w_off[:, u * 96:(u + 1) * 96, k // 3, k % 3].T
    wdeft = np.zeros((128, 1728), np.float32)
    for g in range(G):
        rb = 64 * (g % 2)
        for k in range(KK):
            wdeft[rb: rb + 48, (g * 9 + k) * 48:(g * 9 + k + 1) * 48] = \
                w_def[g * 48:(g + 1) * 48, :, k // 3, k % 3].T

    ident = np.eye(128, dtype=np.float32)
    maskh = np.eye(128, dtype=np.float16)
    selfold = np.zeros((128, 512), np.float32)
    for phi in range(8):
        for j in range(64):
            selfold[16 * phi + (j % 16), phi * 64 + j] = 1.0
    boff = np.asarray(b_off, np.float32).reshape(18, 1)

    p = np.arange(128)
    blkv = np.arange(16)
    baseX = ((p % 64)[:, None, None] - 1 + kx[None, None, :] + PADG
             + np.zeros((1, 16, 1))).astype(np.float32)

    in_maps = []
    for core in range(8):
        b, half = core // 2, core % 2
        h0 = half * HP
        hloc = h0 + blkv[None, :, None] * 2 + (p[:, None, None] // 64)
        baseY = (hloc - 1 + ky[None, None, :] + PADG).astype(np.float32)
        xc = np.zeros((2, 96, 34, 66), np.float32)
        win = xc_full[b, :, h0:h0 + 34, :]          # rows h0-1..h0+32 padded
        xc[0] = win[0:96]
        xc[1] = win[96:192]
        in_maps.append({
            "xT": xT[b], "xc": xc, "wofft": wofft, "boff": boff,
            "wdeft": wdeft, "ident": ident, "maskh": maskh,
            "selfold": selfold,
            "baseY": baseY, "baseX": np.ascontiguousarray(baseX),
        })
    return in_maps


def kernel(x, w_off, b_off, w_def):
    if "nc" not in _CACHE:
        _CACHE["nc"] = _build_nc()
    nc = _CACHE["nc"]
    in_maps = _host_inputs(np.asarray(x), np.asarray(w_off),
                           np.asarray(b_off), np.asarray(w_def))
    res = run_bass_kernel_spmd(nc, in_maps, core_ids=list(range(8)))
    out = np.zeros((B, C, H, W), np.float32)
    for core in range(8):
        b, half = core // 2, core % 2
        out[b, :, half * HP:(half + 1) * HP, :] = \
            res.results[core]["out"].reshape(C, HP, W)
    return out

